# revision 1
# baseline (speedup 1.0000x reference)
"""Trainium2 Bass kernel for a dense transformer block.

Reference computation (per batch element):
    y  = Attention(LN1(x)) ; x = x + y
    x  = x + MLP(LN2(x))
with B=8, N=1024, C=768, H=12 heads, head_dim=64, HIDDEN=3072, fp32 I/O.

Sharding: data-parallel over B across the 8 NeuronCores — each core runs the
full block on one (1024, 768) batch element with replicated weights. No
collectives.

Per-core design notes:
  * Matmul operands are bf16 (weights pre-cast on host); PSUM accumulation and
    the residual stream / layernorm statistics stay fp32.
  * Activations are kept token-major ([tokens->partitions, feats->free]) for
    layernorm + residuals, and feature-major (x_lnT) as the matmul lhsT /
    rhs. The feature-major copies are produced with DMA transposes (bf16).
  * Attention computes S^T = K^T.T @ Q^T per (head, key-tile) so softmax
    probabilities land with k_tokens on partitions, which is exactly the
    layout the attention@V matmul needs as rhs. Softmax denominators come for
    free from a ones-column appended to the (token-major) V tile: the AV
    matmul's 65th output row is the per-query sum of exp-scores.
  * The 1/8 attention scale is folded into the Exp activation's scale input;
    max-subtraction is skipped (scores for this problem are < ~2 in
    magnitude, far from exp overflow).
"""

import os

import numpy as np
import ml_dtypes

import concourse.bass as bass
import concourse.bacc as bacc
import concourse.mybir as mybir
import concourse.tile as tile
from concourse import bass_utils

# Model dims (hardcoded per the problem spec).
B = 8
N = 1024  # tokens
C = 768  # model dim
H = 12  # heads
HD = 64  # head dim
HID = 3072  # mlp hidden
EPS = 1e-5
P = 128  # SBUF partitions

NT = N // P  # 8 token tiles
KC = C // P  # 6 contraction tiles over C
KH = HID // P  # 24 contraction tiles over HIDDEN

F32 = mybir.dt.float32
BF16 = mybir.dt.bfloat16
AF = mybir.ActivationFunctionType
ALU = mybir.AluOpType

_cache = {}

# CoreSim doesn't implement the Gelu activation table; when True the builder
# emits a tanh-approximation decomposition instead (dev/validation only).
SIM_GELU = False


def _build(flags):
    """Trace the per-core Bass program. `flags` gates optional bias/gain work."""
    (use_bqkv, use_g1, use_beta1, use_g2, use_beta2, use_bfc1, use_bproj,
     use_bfc2) = flags

    nc = bacc.Bacc("TRN2", target_bir_lowering=False, debug=False)

    x_d = nc.dram_tensor("x", [N, C], F32, kind="ExternalInput")
    wqkv_d = nc.dram_tensor("wqkv", [C, 3 * C], BF16, kind="ExternalInput")
    wproj_d = nc.dram_tensor("wproj", [C, C], BF16, kind="ExternalInput")
    wfc1_d = nc.dram_tensor("wfc1", [C, HID], BF16, kind="ExternalInput")
    wfc2_d = nc.dram_tensor("wfc2", [HID, C], BF16, kind="ExternalInput")
    out_d = nc.dram_tensor("out", [N, C], F32, kind="ExternalOutput")

    opt_d = {}
    for name, use, shape in (
        ("bqkv", use_bqkv, [3 * C]),
        ("g1", use_g1, [C]),
        ("beta1", use_beta1, [C]),
        ("g2", use_g2, [C]),
        ("beta2", use_beta2, [C]),
        ("bfc1", use_bfc1, [HID]),
        ("bproj", use_bproj, [C]),
        ("bfc2", use_bfc2, [C]),
    ):
        if use:
            opt_d[name] = nc.dram_tensor(name, shape, F32, kind="ExternalInput")

    def bcast_from_dram(pool, ap_1d, n):
        """[n] DRAM vector -> [P, n] SBUF tile replicated on every partition."""
        t = pool.tile([P, n], F32, name=f"bc_{ap_1d.tensor.name}")
        src = bass.AP(tensor=ap_1d.tensor, offset=ap_1d.offset,
                      ap=[[0, P]] + list(ap_1d.ap))
        nc.sync.dma_start(out=t, in_=src)
        return t

    with tile.TileContext(nc) as tc:
        persist = tc.alloc_tile_pool(name="persist", bufs=1, side="left")
        psum = tc.alloc_tile_pool(name="psum", bufs=1, space="PSUM")
        dram = tc.alloc_tile_pool(name="dram", bufs=2, space="DRAM")

        # Residual stream, token-major; updated in place through the block.
        x_sb = persist.tile([P, NT, C], F32)
        for t in range(NT):
            nc.sync.dma_start(out=x_sb[:, t, :], in_=x_d.ap()[t * P:(t + 1) * P, :])
        eps_t = persist.tile([P, 1], F32)
        nc.vector.memset(eps_t, EPS)

        # Identity (bf16, embedded in the NEFF) for PE-based transposes.
        ident_d = nc.inline_tensor(np.eye(P, dtype=ml_dtypes.bfloat16), "ident")
        ident = persist.tile([P, P], BF16)
        nc.sync.dma_start(out=ident, in_=ident_d.ap())

        g_beta = {}
        for name, n in (("g1", C), ("beta1", C), ("g2", C), ("beta2", C),
                        ("bproj", C), ("bfc2", C)):
            if name in opt_d:
                g_beta[name] = bcast_from_dram(persist, opt_d[name].ap(), n)
        bqkv_sb = None
        if "bqkv" in opt_d:
            bqkv_sb = persist.tile([P, 3 * C // P], F32)
            nc.sync.dma_start(out=bqkv_sb,
                              in_=opt_d["bqkv"].ap().rearrange("(m p) -> p m", p=P))
        bfc1_sb = None
        if "bfc1" in opt_d:
            bfc1_sb = persist.tile([P, KH], F32)
            nc.sync.dma_start(out=bfc1_sb,
                              in_=opt_d["bfc1"].ap().rearrange("(m p) -> p m", p=P))

        # ---------------------------------------------------------------
        # Phase 1: LN1 (token-major) -> x_lnT (feature-major bf16), weights
        # ---------------------------------------------------------------
        p1 = tc.alloc_tile_pool(name="p1", bufs=1, side="left")
        ln1 = tc.alloc_tile_pool(name="ln1", bufs=3, side="left")

        wqkv_sb = p1.tile([P, KC, 3 * C], BF16)
        for ko in range(KC):
            nc.sync.dma_start(out=wqkv_sb[:, ko, :],
                              in_=wqkv_d.ap()[ko * P:(ko + 1) * P, :])

        xlnT = p1.tile([P, KC, N], BF16)

        def layernorm_tile(pool, x_ap, g_sb, beta_sb, name):
            """x_ap: [P, C] fp32 token-major -> returns [P, C] bf16 tile."""
            stats = pool.tile([P, 3, 6], F32, tag=f"{name}_st", bufs=3)
            xr = x_ap.rearrange("p (s f) -> p s f", f=256)
            for s in range(3):
                nc.vector.bn_stats(out=stats[:, s, :], in_=xr[:, s, :])
            mv = pool.tile([P, 2], F32, tag=f"{name}_mv", bufs=3)
            nc.vector.bn_aggr(out=mv, in_=stats)
            rstd = pool.tile([P, 1], F32, tag=f"{name}_rs", bufs=3)
            nc.scalar.activation(out=rstd, in_=mv[:, 1:2], func=AF.Sqrt,
                                 bias=eps_t, scale=1.0)
            nc.vector.reciprocal(out=rstd, in_=rstd)
            xln = pool.tile([P, C], BF16, tag=f"{name}_xln", bufs=3)
            nc.vector.tensor_scalar(out=xln, in0=x_ap, scalar1=mv[:, 0:1],
                                    scalar2=rstd, op0=ALU.subtract, op1=ALU.mult)
            if g_sb is not None:
                nc.vector.tensor_mul(out=xln, in0=xln, in1=g_sb)
            if beta_sb is not None:
                nc.vector.tensor_add(out=xln, in0=xln, in1=beta_sb)
            return xln

        def transpose_to(xln, dstT, t):
            """[P, C] token-major tile -> dstT[:, :, t*P:(t+1)*P] feature-major."""
            for c in range(KC):
                tps = psum.tile([P, P], BF16, tag="av", bufs=4, name="tps")
                nc.tensor.transpose(tps, xln[:, c * P:(c + 1) * P], ident)
                nc.scalar.copy(out=dstT[:, c, t * P:(t + 1) * P], in_=tps)

        for t in range(NT):
            xln = layernorm_tile(ln1, x_sb[:, t, :], g_beta.get("g1"),
                                 g_beta.get("beta1"), "ln1")
            transpose_to(xln, xlnT, t)

        # ---------------------------------------------------------------
        # Phase 2: qkv projections.
        #   q^T,k^T feature-major: [2C, N] as 12 tiles of [128, N]
        #   V token-major with ones column: V_aug [P, NT, H, HD+1]
        # ---------------------------------------------------------------
        p2 = tc.alloc_tile_pool(name="p2", bufs=1, side="right")
        qkT = p2.tile([P, 2 * KC, N], BF16)
        v_aug = p2.tile([P, NT, H, HD + 1], BF16)
        nc.vector.memset(v_aug[:, :, :, HD:HD + 1], 1.0)

        # q^T / k^T: out^T[m-block, tokens] = wqkv[:, m-block].T @ x_ln^T
        for m in range(2 * KC):
            for n0 in range(0, N, 512):
                ps = psum.tile([P, 512], F32, tag="mm", bufs=4, name="ps_mm")
                for ko in range(KC):
                    nc.tensor.matmul(ps, wqkv_sb[:, ko, m * P:(m + 1) * P],
                                     xlnT[:, ko, n0:n0 + 512],
                                     start=(ko == 0), stop=(ko == KC - 1))
                if bqkv_sb is not None:
                    nc.any.tensor_scalar_add(qkT[:, m, n0:n0 + 512], ps,
                                             bqkv_sb[:, m:m + 1])
                else:
                    nc.vector.tensor_copy(out=qkT[:, m, n0:n0 + 512], in_=ps)

        # V token-major: V[tok-tile, vfeats] = x_ln @ wqkv[:, 2C:3C]
        for t in range(NT):
            for j, (n0, nn) in enumerate(((0, 512), (512, 256))):
                ps = psum.tile([P, 512], F32, tag="mm", bufs=4, name="ps_mm")[:, :nn]
                for ko in range(KC):
                    nc.tensor.matmul(ps, xlnT[:, ko, t * P:(t + 1) * P],
                                     wqkv_sb[:, ko, 2 * C + n0:2 * C + n0 + nn],
                                     start=(ko == 0), stop=(ko == KC - 1))
                # scatter heads into the 65-strided V_aug layout
                nh = nn // HD
                dst = v_aug[:, t, j * 8:j * 8 + nh, 0:HD]
                if bqkv_sb is not None:
                    # per-feature bias along free dim: use broadcast tile
                    bq = g_beta.get("bqkv_v")
                    if bq is None:
                        bq = bcast_from_dram(persist, opt_d["bqkv"].ap()[2 * C:3 * C], C)
                        g_beta["bqkv_v"] = bq
                    nc.any.tensor_add(out=dst,
                                      in0=ps.rearrange("p (h d) -> p h d", d=HD),
                                      in1=bq[:, n0:n0 + nn].rearrange(
                                          "p (h d) -> p h d", d=HD))
                else:
                    nc.vector.tensor_copy(
                        out=dst, in_=ps.rearrange("p (h d) -> p h d", d=HD))

        ln1.release()
        p1.release()

        # ---------------------------------------------------------------
        # Phase 3: attention, head by head.
        # ---------------------------------------------------------------
        p3 = tc.alloc_tile_pool(name="p3", bufs=1, side="left")
        att = tc.alloc_tile_pool(name="att", bufs=1, side="left")
        attnT = p3.tile([P, KC, N], BF16)
        wproj_sb = p3.tile([P, KC, C], BF16)
        for ko in range(KC):
            nc.sync.dma_start(out=wproj_sb[:, ko, :],
                              in_=wproj_d.ap()[ko * P:(ko + 1) * P, :])

        # Heads are processed in pairs with burst scheduling: all 16 S^T
        # matmuls of a pair are emitted back-to-back (one long PE burst, with
        # the Exp evictions trailing on the Scalar engine), then all 32 AV
        # matmuls. Long uninterrupted bursts keep the PE's HAM clock-gate at
        # full rate; fine-grained S->exp->AV interleaving leaves it throttled.
        for i in range(H // 2):
            es_store = {}
            for h in (2 * i, 2 * i + 1):
                pb = (h % 2) * HD
                qT = qkT[pb:pb + HD, h // 2, :]
                kT = qkT[pb:pb + HD, KC + h // 2, :]
                for kt in range(NT):
                    es = att.tile([P, N], BF16, tag="es", bufs=18,
                                  name=f"es_{h}_{kt}")
                    for j in range(2):
                        sps = psum.tile([P, 512], F32, tag="mm", bufs=4,
                                        name=f"s{j}_{h}_{kt}")
                        nc.tensor.matmul(sps, kT[:, kt * P:(kt + 1) * P],
                                         qT[:, j * 512:(j + 1) * 512],
                                         start=True, stop=True)
                        nc.scalar.activation(out=es[:, j * 512:(j + 1) * 512],
                                             in_=sps, func=AF.Exp, scale=0.125)
                    es_store[h, kt] = es

            avs = {}
            for h in (2 * i, 2 * i + 1):
                avs[h] = [psum.tile([HD + 1, 512], F32, tag="av", bufs=4,
                                    name=f"av{j}_{h}") for j in range(2)]
                for kt in range(NT):
                    for j in range(2):
                        nc.tensor.matmul(avs[h][j], v_aug[:, kt, h, :],
                                         es_store[h, kt][:, j * 512:(j + 1) * 512],
                                         start=(kt == 0), stop=(kt == NT - 1))

            for h in (2 * i, 2 * i + 1):
                pb = (h % 2) * HD
                av = avs[h]
                # Evict accumulators to SBUF right away (fast full-width DVE
                # copies) so the PSUM banks free up for the next pair.
                av_sb = att.tile([HD + 1, N], F32, tag="avsb", bufs=3,
                                 name=f"avsb_{h}")
                for j in range(2):
                    nc.scalar.copy(out=av_sb[:, j * 512:(j + 1) * 512],
                                   in_=av[j])
                # Softmax denominators: row HD holds sum_k exp(S). Broadcast
                # to partitions 0..HD-1 via a DRAM bounce (step-0 partition
                # reads are only legal from DRAM), then reciprocal at full
                # partition width and normalize.
                rdram = dram.tile([1, N], F32, tag="rdram", bufs=2)
                nc.sync.dma_start(out=rdram, in_=av_sb[HD:HD + 1, :])
                rbc = att.tile([HD, N], F32, tag="rbc", bufs=2, name=f"rbc{h}")
                rd = rdram[0, :]
                rbc_src = bass.AP(tensor=rd.tensor, offset=rd.offset,
                                  ap=[[0, HD]] + list(rd.ap))
                nc.sync.dma_start(out=rbc, in_=rbc_src)
                nc.vector.reciprocal(out=rbc, in_=rbc)
                bounce = att.tile([HD, N], BF16, tag="bounce", bufs=2,
                                  name=f"bounce{h}")
                nc.vector.tensor_mul(out=bounce, in0=av_sb[0:HD, :], in1=rbc)
                nc.sync.dma_start(out=attnT[pb:pb + HD, h // 2, :], in_=bounce)

        p2.release()

        # ---------------------------------------------------------------
        # Phase 4: proj + residual, LN2 -> x2_lnT
        # ---------------------------------------------------------------
        p4 = tc.alloc_tile_pool(name="p4", bufs=1, side="right")
        ln2 = tc.alloc_tile_pool(name="ln2", bufs=3, side="right")
        x2lnT = p4.tile([P, KC, N], BF16)
        wfc1_sb = p4.tile([P, KC, HID], BF16)
        for ko in range(KC):
            nc.sync.dma_start(out=wfc1_sb[:, ko, :],
                              in_=wfc1_d.ap()[ko * P:(ko + 1) * P, :])

        for t in range(NT):
            for n0, nn in ((0, 512), (512, 256)):
                ps = psum.tile([P, 512], F32, tag="mm", bufs=4, name="ps_mm")[:, :nn]
                for ko in range(KC):
                    nc.tensor.matmul(ps, attnT[:, ko, t * P:(t + 1) * P],
                                     wproj_sb[:, ko, n0:n0 + nn],
                                     start=(ko == 0), stop=(ko == KC - 1))
                xs = x_sb[:, t, n0:n0 + nn]
                nc.vector.tensor_add(out=xs, in0=xs, in1=ps)
                if "bproj" in g_beta:
                    nc.vector.tensor_add(out=xs, in0=xs,
                                         in1=g_beta["bproj"][:, n0:n0 + nn])
            xln = layernorm_tile(ln2, x_sb[:, t, :], g_beta.get("g2"),
                                 g_beta.get("beta2"), "ln2")
            transpose_to(xln, x2lnT, t)

        att.release()
        p3.release()

        # ---------------------------------------------------------------
        # Phase 5: fc1 + gelu -> h^T (feature-major bf16)
        # ---------------------------------------------------------------
        p5 = tc.alloc_tile_pool(name="p5", bufs=1, side="left")
        hT = p5.tile([P, KH, N], BF16)
        wfc2_sb = p5.tile([P, KH, C], BF16)
        for ko in range(KH):
            nc.sync.dma_start(out=wfc2_sb[:, ko, :],
                              in_=wfc2_d.ap()[ko * P:(ko + 1) * P, :])

        for m in range(KH):
            for n0 in range(0, N, 512):
                ps = psum.tile([P, 512], F32, tag="mm", bufs=4, name="ps_mm")
                for ko in range(KC):
                    nc.tensor.matmul(ps, wfc1_sb[:, ko, m * P:(m + 1) * P],
                                     x2lnT[:, ko, n0:n0 + 512],
                                     start=(ko == 0), stop=(ko == KC - 1))
                bias = bfc1_sb[:, m:m + 1] if bfc1_sb is not None else 0.0
                if not SIM_GELU:
                    nc.scalar.activation(out=hT[:, m, n0:n0 + 512], in_=ps,
                                         func=AF.Gelu, bias=bias, scale=1.0)
                else:
                    # gelu(x) ~= 0.5 x (1 + tanh(sqrt(2/pi)(x + 0.044715 x^3)))
                    a = ln2.tile([P, 512], F32, tag="g_a", bufs=2)
                    nc.scalar.activation(out=a, in_=ps, func=AF.Copy,
                                         bias=0.0, scale=1.0)
                    if bfc1_sb is not None:
                        nc.vector.tensor_scalar_add(a, a, bfc1_sb[:, m:m + 1])
                    u = ln2.tile([P, 512], F32, tag="g_u", bufs=2)
                    nc.vector.tensor_mul(out=u, in0=a, in1=a)
                    nc.vector.tensor_mul(out=u, in0=u, in1=a)
                    nc.vector.tensor_scalar_mul(u, u, 0.044715)
                    nc.vector.tensor_add(out=u, in0=u, in1=a)
                    nc.scalar.activation(out=u, in_=u, func=AF.Tanh,
                                         bias=0.0, scale=0.7978845608028654)
                    nc.vector.tensor_scalar_add(u, u, 1.0)
                    nc.vector.tensor_scalar_mul(a, a, 0.5)
                    nc.vector.tensor_mul(out=hT[:, m, n0:n0 + 512],
                                         in0=a, in1=u)

        ln2.release()
        p4.release()

        # ---------------------------------------------------------------
        # Phase 6: fc2 + residual -> out
        # ---------------------------------------------------------------
        for t in range(NT):
            for n0, nn in ((0, 512), (512, 256)):
                ps = psum.tile([P, 512], F32, tag="mm", bufs=4, name="ps_mm")[:, :nn]
                for ko in range(KH):
                    nc.tensor.matmul(ps, hT[:, ko, t * P:(t + 1) * P],
                                     wfc2_sb[:, ko, n0:n0 + nn],
                                     start=(ko == 0), stop=(ko == KH - 1))
                xs = x_sb[:, t, n0:n0 + nn]
                nc.vector.tensor_add(out=xs, in0=xs, in1=ps)
                if "bfc2" in g_beta:
                    nc.vector.tensor_add(out=xs, in0=xs,
                                         in1=g_beta["bfc2"][:, n0:n0 + nn])
            nc.sync.dma_start(out=out_d.ap()[t * P:(t + 1) * P, :],
                              in_=x_sb[:, t, :])

        p5.release()
        persist.release()
        dram.release()
        psum.release()

    nc.compile()
    return nc


def _prep(inputs):
    """Host-side prep: shard x over B, cast weights to bf16, compute gates."""
    f = {k: np.asarray(v) for k, v in inputs.items()}
    bf = ml_dtypes.bfloat16

    flags = (
        bool(np.any(f["b_qkv"])),
        not np.all(f["g1"] == 1.0),
        bool(np.any(f["beta1"])),
        not np.all(f["g2"] == 1.0),
        bool(np.any(f["beta2"])),
        bool(np.any(f["b_fc1"])),
        bool(np.any(f["b_proj"])),
        bool(np.any(f["b_fc2"])),
    )
    (use_bqkv, use_g1, use_beta1, use_g2, use_beta2, use_bfc1, use_bproj,
     use_bfc2) = flags

    common = {
        "wqkv": np.ascontiguousarray(f["w_qkv"].astype(bf)),
        "wproj": np.ascontiguousarray(f["w_proj"].astype(bf)),
        "wfc1": np.ascontiguousarray(f["w_fc1"].astype(bf)),
        "wfc2": np.ascontiguousarray(f["w_fc2"].astype(bf)),
    }
    for name, key, use in (
        ("bqkv", "b_qkv", use_bqkv), ("g1", "g1", use_g1),
        ("beta1", "beta1", use_beta1), ("g2", "g2", use_g2),
        ("beta2", "beta2", use_beta2), ("bfc1", "b_fc1", use_bfc1),
        ("bproj", "b_proj", use_bproj), ("bfc2", "b_fc2", use_bfc2),
    ):
        if use:
            common[name] = np.ascontiguousarray(f[key].astype(np.float32))

    x = f["x"].astype(np.float32)
    in_maps = [dict(common, x=np.ascontiguousarray(x[i])) for i in range(B)]
    return flags, in_maps


LAST_RESULT = None


def kernel(**inputs):
    global LAST_RESULT
    flags, in_maps = _prep(inputs)
    if flags not in _cache:
        _cache[flags] = _build(flags)
    nc = _cache[flags]
    res = bass_utils.run_bass_kernel_spmd(nc, in_maps, core_ids=list(range(B)))
    LAST_RESULT = res
    out = np.stack([r["out"] for r in res.results], axis=0)
    return out.astype(np.float32)



# revision 17
# speedup vs baseline: 1.1322x; 1.1322x over previous
"""Trainium2 Bass kernel for a dense transformer block.

Reference computation (per batch element):
    y  = Attention(LN1(x)) ; x = x + y
    x  = x + MLP(LN2(x))
with B=8, N=1024, C=768, H=12 heads, head_dim=64, HIDDEN=3072, fp32 I/O.

Sharding: data-parallel over B across the 8 NeuronCores — each core runs the
full block on one (1024, 768) batch element with replicated weights. No
collectives.

Per-core design notes:
  * Matmul operands are bf16 (weights pre-cast on host); PSUM accumulation and
    the residual stream / layernorm statistics stay fp32.
  * Activations are kept token-major for layernorm + residuals, and
    feature-major (x_lnT) as the matmul lhsT / rhs, produced via PE
    transposes.
  * The QKV projections for head-pair i+1 are interleaved into the attention
    compute of head-pair i. Attention alone leaves the PE ~65% busy (gated on
    the scalar engine's Exp), which keeps the PE_HAM activity monitor
    throttled at half clock; the extra matmuls push PE duty near 100% so the
    array runs at 2.4 GHz through the whole attention span.
  * S^T = K^T.T @ Q^T per (head, key-tile) lands softmax scores with k_tokens
    on partitions (the layout attention@V wants as rhs). Exp runs as one
    [128, 1024] scalar-engine instruction over a two-bank PSUM pair. Softmax
    denominators come free from a ones-column appended to V: the AV matmul's
    65th output row is the per-query sum of exp-scores.
  * Softmax normalization: the denominator row is broadcast down 64
    partitions by the (otherwise idle) GPSIMD engine, reciprocal'd with the
    fast approx DVE op, and multiplied into the attention rows. Odd heads
    hop partitions 0:64 -> 64:128 with one SBUF->SBUF DMA.
  * The 1/8 attention scale is folded into the Exp activation's scale input;
    max-subtraction is skipped (scores for this problem are < ~2 in
    magnitude, far from exp overflow).
"""

import numpy as np
import ml_dtypes

import concourse.bass as bass
import concourse.bacc as bacc
import concourse.mybir as mybir
import concourse.tile as tile
from concourse import bass_utils

# Model dims (hardcoded per the problem spec).
B = 8
N = 1024  # tokens
C = 768  # model dim
H = 12  # heads
HD = 64  # head dim
HID = 3072  # mlp hidden
EPS = 1e-5
P = 128  # SBUF partitions

NT = N // P  # 8 token tiles
KC = C // P  # 6 contraction tiles over C
KH = HID // P  # 24 contraction tiles over HIDDEN
NPAIR = H // 2  # 6 head pairs

F32 = mybir.dt.float32
BF16 = mybir.dt.bfloat16
AF = mybir.ActivationFunctionType
ALU = mybir.AluOpType

_cache = {}

# CoreSim doesn't implement the Gelu activation table; debug-only switch so
# the program can be validated in the simulator (with a matching reference).
SIM_GELU_COPY = False


def _build(flags):
    """Trace the per-core Bass program. `flags` gates optional bias/gain work."""
    (use_bqkv, use_g1, use_beta1, use_g2, use_beta2, use_bfc1, use_bproj,
     use_bfc2) = flags

    nc = bacc.Bacc("TRN2", target_bir_lowering=False, debug=False)

    x_d = nc.dram_tensor("x", [N, C], F32, kind="ExternalInput")
    wqkv_d = nc.dram_tensor("wqkv", [C, 3 * C], BF16, kind="ExternalInput")
    wproj_d = nc.dram_tensor("wproj", [C, C], BF16, kind="ExternalInput")
    wfc1_d = nc.dram_tensor("wfc1", [C, HID], BF16, kind="ExternalInput")
    wfc2_d = nc.dram_tensor("wfc2", [HID, C], BF16, kind="ExternalInput")
    out_d = nc.dram_tensor("out", [N, C], F32, kind="ExternalOutput")

    opt_d = {}
    for name, use, shape in (
        ("bqkv", use_bqkv, [3 * C]),
        ("g1", use_g1, [C]),
        ("beta1", use_beta1, [C]),
        ("g2", use_g2, [C]),
        ("beta2", use_beta2, [C]),
        ("bfc1", use_bfc1, [HID]),
        ("bproj", use_bproj, [C]),
        ("bfc2", use_bfc2, [C]),
    ):
        if use:
            opt_d[name] = nc.dram_tensor(name, shape, F32, kind="ExternalInput")

    def bcast_from_dram(pool, ap_1d, n):
        """[n] DRAM vector -> [P, n] SBUF tile replicated on every partition."""
        t = pool.tile([P, n], F32, name=f"bc_{ap_1d.tensor.name}")
        src = bass.AP(tensor=ap_1d.tensor, offset=ap_1d.offset,
                      ap=[[0, P]] + list(ap_1d.ap))
        nc.sync.dma_start(out=t, in_=src)
        return t

    with tile.TileContext(nc) as tc:
        persist = tc.alloc_tile_pool(name="persist", bufs=1, side="left")
        psum = tc.alloc_tile_pool(name="psum", bufs=1, space="PSUM")
        dram = tc.alloc_tile_pool(name="dram", bufs=2, space="DRAM")

        # Residual stream, token-major; updated in place through the block.
        # Two DMAs on separate queues to halve the load latency.
        x_sb = persist.tile([P, NT, C], F32)
        x_r = x_d.ap().rearrange("(t p) c -> p t c", p=P)
        nc.sync.dma_start(out=x_sb[:, 0:NT // 2, :], in_=x_r[:, 0:NT // 2, :])
        nc.gpsimd.dma_start(out=x_sb[:, NT // 2:NT, :], in_=x_r[:, NT // 2:NT, :])

        eps_t = persist.tile([P, 1], F32)
        nc.vector.memset(eps_t, EPS)

        # Identity (bf16, embedded in the NEFF) for PE-based transposes.
        ident_d = nc.inline_tensor(np.eye(P, dtype=ml_dtypes.bfloat16), "ident")
        ident = persist.tile([P, P], BF16)
        nc.scalar.dma_start(out=ident, in_=ident_d.ap())

        g_beta = {}
        for name, n in (("g1", C), ("beta1", C), ("g2", C), ("beta2", C),
                        ("bproj", C), ("bfc2", C)):
            if name in opt_d:
                g_beta[name] = bcast_from_dram(persist, opt_d[name].ap(), n)
        bqkv_sb = None
        if "bqkv" in opt_d:
            bqkv_sb = persist.tile([P, 3 * C // P], F32)
            nc.sync.dma_start(out=bqkv_sb,
                              in_=opt_d["bqkv"].ap().rearrange("(m p) -> p m", p=P))
        bfc1_sb = None
        if "bfc1" in opt_d:
            bfc1_sb = persist.tile([P, KH], F32)
            nc.sync.dma_start(out=bfc1_sb,
                              in_=opt_d["bfc1"].ap().rearrange("(m p) -> p m", p=P))

        # ---------------------------------------------------------------
        # Phase 1: LN1 (token-major) -> x_lnT (feature-major bf16), weights
        # ---------------------------------------------------------------
        p1 = tc.alloc_tile_pool(name="p1", bufs=1, side="left")
        p3 = tc.alloc_tile_pool(name="p3", bufs=1, side="left")
        ln1 = tc.alloc_tile_pool(name="ln1", bufs=3, side="left")

        wqkv_sb = p1.tile([P, KC, 3 * C], BF16)
        nc.scalar.dma_start(out=wqkv_sb,
                            in_=wqkv_d.ap().rearrange("(k p) m -> p k m", p=P))

        xlnT = p1.tile([P, KC, N], BF16)

        attnT = p3.tile([P, KC, N], BF16)
        wproj_sb = p3.tile([P, KC, C], BF16)
        nc.sync.dma_start(out=wproj_sb,
                          in_=wproj_d.ap().rearrange("(k p) m -> p k m", p=P))

        def layernorm_tile(pool, x_ap, g_sb, beta_sb, name):
            """x_ap: [P, C] fp32 token-major -> returns [P, C] bf16 tile."""
            stats = pool.tile([P, 3, 6], F32, tag=f"{name}_st", bufs=3)
            xr = x_ap.rearrange("p (s f) -> p s f", f=256)
            for s in range(3):
                nc.vector.bn_stats(out=stats[:, s, :], in_=xr[:, s, :])
            mv = pool.tile([P, 2], F32, tag=f"{name}_mv", bufs=3)
            nc.vector.bn_aggr(out=mv, in_=stats)
            rstd = pool.tile([P, 1], F32, tag=f"{name}_rs", bufs=3)
            nc.scalar.activation(out=rstd, in_=mv[:, 1:2], func=AF.Sqrt,
                                 bias=eps_t, scale=1.0)
            nc.vector.reciprocal(out=rstd, in_=rstd)
            xln = pool.tile([P, C], BF16, tag=f"{name}_xln", bufs=3)
            nc.vector.tensor_scalar(out=xln, in0=x_ap, scalar1=mv[:, 0:1],
                                    scalar2=rstd, op0=ALU.subtract, op1=ALU.mult)
            if g_sb is not None:
                nc.vector.tensor_mul(out=xln, in0=xln, in1=g_sb)
            if beta_sb is not None:
                nc.vector.tensor_add(out=xln, in0=xln, in1=beta_sb)
            return xln

        def transpose_to(xln, dstT, t):
            """[P, C] token-major tile -> dstT[:, :, t*P:(t+1)*P] feature-major.

            Evictions ride the scalar engine: it is idle in the layernorm
            phases, and the DVE (which carries the LN math) is not."""
            for c in range(KC):
                tps = psum.tile([P, P], BF16, tag="mm", bufs=2, name="tps")
                nc.tensor.transpose(tps, xln[:, c * P:(c + 1) * P], ident)
                nc.scalar.copy(out=dstT[:, c, t * P:(t + 1) * P], in_=tps)

        for t in range(NT):
            xln = layernorm_tile(ln1, x_sb[:, t, :], g_beta.get("g1"),
                                 g_beta.get("beta1"), "ln1")
            transpose_to(xln, xlnT, t)

        # ---------------------------------------------------------------
        # Phase 2 prologue: V for all heads (token-major, ones column) and
        # the q/k projections for head-pair 0. q/k for pair i+1 are emitted
        # inside pair i's attention loop below.
        #   q^T,k^T feature-major: [2C, N] as 12 tiles of [128, N]
        #   V token-major with ones column: V_aug [P, NT, H, HD+1]
        # ---------------------------------------------------------------
        p2 = tc.alloc_tile_pool(name="p2", bufs=1, side="right")
        qkT = p2.tile([P, 2 * KC, N], BF16)
        v_aug = p2.tile([P, NT, H, HD + 1], BF16)
        nc.vector.memset(v_aug[:, :, :, HD:HD + 1], 1.0)

        def emit_qk_chain(m, n0):
            """qkT[m-block, n0:n0+512] = (wqkv[:, m-block].T @ x_ln^T) chunk."""
            ps = psum.tile([P, 512], F32, tag="mm", bufs=2, name="ps_mm")
            for ko in range(KC):
                nc.tensor.matmul(ps, wqkv_sb[:, ko, m * P:(m + 1) * P],
                                 xlnT[:, ko, n0:n0 + 512],
                                 start=(ko == 0), stop=(ko == KC - 1))
            if bqkv_sb is not None:
                nc.vector.tensor_scalar_add(qkT[:, m, n0:n0 + 512], ps,
                                            bqkv_sb[:, m:m + 1])
            else:
                nc.vector.tensor_copy(out=qkT[:, m, n0:n0 + 512], in_=ps)

        for m in (0, KC):
            for n0 in (0, 512):
                emit_qk_chain(m, n0)

        # V token-major: V[tok-tile, vfeats] = x_ln @ wqkv[:, 2C:3C]
        for t in range(NT):
            for j, (n0, nn) in enumerate(((0, 512), (512, 256))):
                ps = psum.tile([P, 512], F32, tag="mm", bufs=2, name="ps_mm")[:, :nn]
                for ko in range(KC):
                    nc.tensor.matmul(ps, xlnT[:, ko, t * P:(t + 1) * P],
                                     wqkv_sb[:, ko, 2 * C + n0:2 * C + n0 + nn],
                                     start=(ko == 0), stop=(ko == KC - 1))
                # scatter heads into the 65-strided V_aug layout
                nh = nn // HD
                dst = v_aug[:, t, j * 8:j * 8 + nh, 0:HD]
                if bqkv_sb is not None:
                    bq = g_beta.get("bqkv_v")
                    if bq is None:
                        bq = bcast_from_dram(persist, opt_d["bqkv"].ap()[2 * C:3 * C], C)
                        g_beta["bqkv_v"] = bq
                    nc.vector.tensor_add(out=dst,
                                         in0=ps.rearrange("p (h d) -> p h d", d=HD),
                                         in1=bq[:, n0:n0 + nn].rearrange(
                                             "p (h d) -> p h d", d=HD))
                else:
                    nc.vector.tensor_copy(
                        out=dst, in_=ps.rearrange("p (h d) -> p h d", d=HD))

        ln1.release()

        # ---------------------------------------------------------------
        # Phase 3: attention, head-pair by head-pair, with next pair's q/k
        # matmuls interleaved to keep the PE dense (HAM stays un-throttled).
        # ---------------------------------------------------------------
        att = tc.alloc_tile_pool(name="att", bufs=1, side="left")

        # Per head: the S matmuls, the Exp evictions, the AV accumulation
        # (trailing the Exps by one key-tile) and the next pair's q/k
        # projection chains are emitted at key-tile granularity. The PE's
        # in-order queue then alternates S / AV / filler matmuls, staying
        # ~100% busy at exactly the pace the scalar engine produces Exps —
        # dense PE activity keeps the HAM clock gate at the full 2.4 GHz.
        for i in range(NPAIR):
            # The 4 q/k chains for pair i+1, doled out two per head.
            fill = []
            if i + 1 < NPAIR:
                fill = [(i + 1, 0), (i + 1, 512), (KC + i + 1, 0),
                        (KC + i + 1, 512)]
            for h in (2 * i, 2 * i + 1):
                pb = (h % 2) * HD
                qT = qkT[pb:pb + HD, i, :]
                kT = qkT[pb:pb + HD, KC + i, :]
                es_store = {}
                av = [psum.tile([HD + 1, 512], F32, tag="av", bufs=2,
                                name=f"av{j}_{h}") for j in range(2)]

                def emit_av(kt):
                    for j in range(2):
                        nc.tensor.matmul(av[j], v_aug[:, kt, h, :],
                                         es_store[kt][:, j * 512:(j + 1) * 512],
                                         start=(kt == 0), stop=(kt == NT - 1))

                for kt in range(NT):
                    sps = psum.tile([P, 2, 512], F32, tag="s", bufs=2,
                                    name=f"s_{h}_{kt}")
                    for j in range(2):
                        nc.tensor.matmul(sps[:, j, :], kT[:, kt * P:(kt + 1) * P],
                                         qT[:, j * 512:(j + 1) * 512],
                                         start=True, stop=True)
                    es = att.tile([P, N], BF16, tag="es", bufs=8,
                                  name=f"es_{h}_{kt}")
                    nc.scalar.activation(out=es,
                                         in_=sps.rearrange("p a b -> p (a b)"),
                                         func=AF.Exp, scale=0.125)
                    es_store[kt] = es
                    if kt > 0:
                        emit_av(kt - 1)
                    if kt in (2, 5) and fill:
                        emit_qk_chain(*fill.pop(0))
                emit_av(NT - 1)

                # Evict accumulators to SBUF right away so the PSUM banks
                # free up for the next head's chains.
                av_sb = att.tile([HD + 1, N], F32, tag="avsb", bufs=3,
                                 name=f"avsb_{h}")
                for j in range(2):
                    nc.vector.tensor_copy(out=av_sb[:, j * 512:(j + 1) * 512],
                                          in_=av[j])
                # Softmax denominators: row HD holds sum_k exp(S). Broadcast
                # to partitions 0..HD-1 via a DRAM bounce (step-0 partition
                # reads are only legal from DRAM), then (approx) reciprocal
                # at full partition width.
                rdram = dram.tile([1, N], F32, tag="rdram", bufs=2)
                nc.gpsimd.dma_start(out=rdram, in_=av_sb[HD:HD + 1, :])
                rbc = att.tile([HD, N], F32, tag="rbc", bufs=2, name=f"rbc{h}")
                rd = rdram[0, :]
                rbc_src = bass.AP(tensor=rd.tensor, offset=rd.offset,
                                  ap=[[0, HD]] + list(rd.ap))
                nc.gpsimd.dma_start(out=rbc, in_=rbc_src)
                nc.vector.reciprocal_approx_fast(out=rbc, in_=rbc)
                if h % 2 == 0:
                    nc.vector.tensor_mul(out=attnT[0:HD, i, :],
                                         in0=av_sb[0:HD, :], in1=rbc)
                else:
                    # Odd heads land on partitions 64:128 of attnT — engines
                    # can't shift partitions, so bounce through one DMA.
                    bounce = att.tile([HD, N], BF16, tag="bounce", bufs=2,
                                      name=f"bounce{h}")
                    nc.vector.tensor_mul(out=bounce, in0=av_sb[0:HD, :], in1=rbc)
                    nc.gpsimd.dma_start(out=attnT[HD:P, i, :], in_=bounce)

        att.release()
        p2.release()

        # ---------------------------------------------------------------
        # Phase 4: proj + residual, LN2 -> x2_lnT
        # ---------------------------------------------------------------
        p4 = tc.alloc_tile_pool(name="p4", bufs=1, side="right")
        ln2 = tc.alloc_tile_pool(name="ln2", bufs=3, side="right")
        x2lnT = p4.tile([P, KC, N], BF16)
        wfc1_sb = p4.tile([P, KC, HID], BF16)
        wfc1_r = wfc1_d.ap().rearrange("(k p) m -> p k m", p=P)
        nc.sync.dma_start(out=wfc1_sb[:, 0:KC // 2, :], in_=wfc1_r[:, 0:KC // 2, :])
        nc.gpsimd.dma_start(out=wfc1_sb[:, KC // 2:KC, :],
                            in_=wfc1_r[:, KC // 2:KC, :])

        for t in range(NT):
            for n0, nn in ((0, 512), (512, 256)):
                ps = psum.tile([P, 512], F32, tag="mm", bufs=2, name="ps_mm")[:, :nn]
                for ko in range(KC):
                    nc.tensor.matmul(ps, attnT[:, ko, t * P:(t + 1) * P],
                                     wproj_sb[:, ko, n0:n0 + nn],
                                     start=(ko == 0), stop=(ko == KC - 1))
                xs = x_sb[:, t, n0:n0 + nn]
                nc.vector.tensor_add(out=xs, in0=xs, in1=ps)
                if "bproj" in g_beta:
                    nc.vector.tensor_add(out=xs, in0=xs,
                                         in1=g_beta["bproj"][:, n0:n0 + nn])
            xln = layernorm_tile(ln2, x_sb[:, t, :], g_beta.get("g2"),
                                 g_beta.get("beta2"), "ln2")
            transpose_to(xln, x2lnT, t)

        p3.release()
        p1.release()

        # ---------------------------------------------------------------
        # Phase 5: fc1 + gelu -> h^T (feature-major bf16)
        # ---------------------------------------------------------------
        p5 = tc.alloc_tile_pool(name="p5", bufs=1, side="left")
        hT = p5.tile([P, KH, N], BF16)
        wfc2_sb = p5.tile([P, KH, C], BF16)
        wfc2_r = wfc2_d.ap().rearrange("(k p) m -> p k m", p=P)
        nc.sync.dma_start(out=wfc2_sb[:, 0:KH // 2, :], in_=wfc2_r[:, 0:KH // 2, :])
        nc.gpsimd.dma_start(out=wfc2_sb[:, KH // 2:KH, :],
                            in_=wfc2_r[:, KH // 2:KH, :])

        for m in range(KH):
            sps = psum.tile([P, 2, 512], F32, tag="s", bufs=2, name="ps_fc1")
            for j in range(2):
                for ko in range(KC):
                    nc.tensor.matmul(sps[:, j, :],
                                     wfc1_sb[:, ko, m * P:(m + 1) * P],
                                     x2lnT[:, ko, j * 512:(j + 1) * 512],
                                     start=(ko == 0), stop=(ko == KC - 1))
            bias = bfc1_sb[:, m:m + 1] if bfc1_sb is not None else 0.0
            nc.scalar.activation(out=hT[:, m, :],
                                 in_=sps.rearrange("p a b -> p (a b)"),
                                 func=AF.Copy if SIM_GELU_COPY else AF.Gelu,
                                 bias=bias, scale=1.0)

        ln2.release()
        p4.release()

        # ---------------------------------------------------------------
        # Phase 6: fc2 + residual -> out
        # ---------------------------------------------------------------
        for t in range(NT):
            for n0, nn in ((0, 512), (512, 256)):
                ps = psum.tile([P, 512], F32, tag="mm", bufs=2, name="ps_mm")[:, :nn]
                for ko in range(KH):
                    nc.tensor.matmul(ps, hT[:, ko, t * P:(t + 1) * P],
                                     wfc2_sb[:, ko, n0:n0 + nn],
                                     start=(ko == 0), stop=(ko == KH - 1))
                xs = x_sb[:, t, n0:n0 + nn]
                nc.vector.tensor_add(out=xs, in0=xs, in1=ps)
                if "bfc2" in g_beta:
                    nc.vector.tensor_add(out=xs, in0=xs,
                                         in1=g_beta["bfc2"][:, n0:n0 + nn])
            nc.sync.dma_start(out=out_d.ap()[t * P:(t + 1) * P, :],
                              in_=x_sb[:, t, :])

        p5.release()
        persist.release()
        dram.release()
        psum.release()

    nc.compile()
    return nc


def _prep(inputs):
    """Host-side prep: shard x over B, cast weights to bf16, compute gates."""
    f = {k: np.asarray(v) for k, v in inputs.items()}
    bf = ml_dtypes.bfloat16

    flags = (
        bool(np.any(f["b_qkv"])),
        not np.all(f["g1"] == 1.0),
        bool(np.any(f["beta1"])),
        not np.all(f["g2"] == 1.0),
        bool(np.any(f["beta2"])),
        bool(np.any(f["b_fc1"])),
        bool(np.any(f["b_proj"])),
        bool(np.any(f["b_fc2"])),
    )
    (use_bqkv, use_g1, use_beta1, use_g2, use_beta2, use_bfc1, use_bproj,
     use_bfc2) = flags

    common = {
        "wqkv": np.ascontiguousarray(f["w_qkv"].astype(bf)),
        "wproj": np.ascontiguousarray(f["w_proj"].astype(bf)),
        "wfc1": np.ascontiguousarray(f["w_fc1"].astype(bf)),
        "wfc2": np.ascontiguousarray(f["w_fc2"].astype(bf)),
    }
    for name, key, use in (
        ("bqkv", "b_qkv", use_bqkv), ("g1", "g1", use_g1),
        ("beta1", "beta1", use_beta1), ("g2", "g2", use_g2),
        ("beta2", "beta2", use_beta2), ("bfc1", "b_fc1", use_bfc1),
        ("bproj", "b_proj", use_bproj), ("bfc2", "b_fc2", use_bfc2),
    ):
        if use:
            common[name] = np.ascontiguousarray(f[key].astype(np.float32))

    x = f["x"].astype(np.float32)
    in_maps = [dict(common, x=np.ascontiguousarray(x[i])) for i in range(B)]
    return flags, in_maps


LAST_RESULT = None


def kernel(**inputs):
    global LAST_RESULT
    flags, in_maps = _prep(inputs)
    if flags not in _cache:
        _cache[flags] = _build(flags)
    nc = _cache[flags]
    res = bass_utils.run_bass_kernel_spmd(nc, in_maps, core_ids=list(range(B)))
    LAST_RESULT = res
    out = np.stack([r["out"] for r in res.results], axis=0)
    return out.astype(np.float32)
